# revision 13
# baseline (speedup 1.0000x reference)
"""Trainium2 Bass kernel for nn_EnhancedBioKANModel (dense_transformer).

Model (B=4096, IN=3072, D=2048, C=1000, 3 blocks), with two crucial
mathematical simplifications:

1. The internal sequence length is 1, so attention's softmax over a single
   key is identically 1.0 and the whole score/top-k/conv machinery cancels:
       attn(h) = (h @ Wv^T + bv) @ Wo^T + bo
2. That attention is then a purely linear map, so the residual attention
   block folds into ONE matrix host-side:
       h <- h + attn(h) = (I + Wo Wv) h + (Wo bv + bo) = Wff h + bf

Per layer:   h = Wff h + bf;  z = relu(LN1(W1 h + b1));  h = h + LN2(W2 z + b2)

Strategy: pure data-parallel over batch (512 rows/core on 8 cores),
feature-major activation layout [D partitions, 512 batch free],
host-pre-transposed weights, fp32r matmuls (full PE rate at N=512),
LayerNorm means folded into host-centered W1/W2, variance via
ScalarE-square + PE ones-reduction, per-batch 1/std broadcast via
K=1 matmul.
"""
from contextlib import ExitStack

import numpy as np

import concourse.bacc as bacc
import concourse.bass as bass
import concourse.mybir as mybir
import concourse.tile as tile
from concourse.bass_utils import run_bass_kernel_spmd

F32 = mybir.dt.float32
F32R = mybir.dt.float32r
AF = mybir.ActivationFunctionType
ALU = mybir.AluOpType

NCORES = 8
B, IN, D, DD, C = 4096, 3072, 2048, 4096, 1000
CP = 1024          # padded num_classes
BC = B // NCORES   # 512 batch per core
NB = 3
EPS = 1e-5
MG = 4             # m-chunks per psum group
KB = 4             # k-chunks per weight DMA (1 MiB transfers, 8 KiB lines)
KBX = 8            # k-chunks per x DMA

# consts packing (columns of [128, NCOL] fp32), per layer:
#   bf(16) b1c(32) g1(32) beta1(32) b2c(16) g2(16) beta2(16)
_LAYER_COLS = 160
_NCOL = 16 + NB * _LAYER_COLS + CP // 128

_cached = None
last_results = None


def _build():
    nc = bacc.Bacc(trn_type="TRN2")

    def wparam(name, K, M):
        # swizzled: [M/512 groups, K/(128*KB) blocks, 128 partitions, KB*512]
        return nc.declare_dram_parameter(
            name, [M // (MG * 128), K // (128 * KB), 128, KB * MG * 128],
            F32R, isOutput=False)

    xt = nc.declare_dram_parameter("xt", [IN, BC], F32R, isOutput=False)
    wit = wparam("wit", IN, D)
    wfft = [wparam(f"wfft{l}", D, D) for l in range(NB)]
    w1t = [wparam(f"w1t{l}", D, DD) for l in range(NB)]
    w2t = [wparam(f"w2t{l}", DD, D) for l in range(NB)]
    woutt = wparam("woutt", D, CP)
    consts = nc.declare_dram_parameter("consts", [128, _NCOL], F32, isOutput=False)
    out_t = nc.declare_dram_parameter("outT", [CP, BC], F32, isOutput=True)

    with tile.TileContext(nc) as tc, ExitStack() as ctx:
        wpool = ctx.enter_context(tc.tile_pool(name="w", bufs=3))
        bigpool = ctx.enter_context(tc.tile_pool(name="big", bufs=1))
        hpool = ctx.enter_context(tc.tile_pool(name="h", bufs=1))
        upool = ctx.enter_context(tc.tile_pool(name="u", bufs=1))
        sqpool = ctx.enter_context(tc.tile_pool(name="sq", bufs=3))
        opool = ctx.enter_context(tc.tile_pool(name="o", bufs=2))
        statpool = ctx.enter_context(tc.tile_pool(name="stat", bufs=2))
        singles = ctx.enter_context(tc.tile_pool(name="singles", bufs=1))
        psum = ctx.enter_context(tc.tile_pool(name="psum", bufs=6, space="PSUM"))
        pstat = ctx.enter_context(tc.tile_pool(name="pstat", bufs=1, space="PSUM"))
        pbc = ctx.enter_context(tc.tile_pool(name="pbc", bufs=1, space="PSUM"))

        # ---- constants ----
        cst = singles.tile([128, _NCOL], F32)
        nc.sync.dma_start(out=cst, in_=consts[:, :])

        ones_f = singles.tile([128, 1], F32)
        nc.vector.memset(ones_f, 1.0)
        ones_col = singles.tile([128, 1], F32R)
        nc.vector.tensor_copy(ones_col[:, :], ones_f[:, :])
        ones_row_f = singles.tile([1, 128], F32)
        nc.vector.memset(ones_row_f, 1.0)
        ones_row = singles.tile([1, 128], F32R)
        nc.vector.tensor_copy(ones_row[:, :], ones_row_f[:, :])
        eps_sb = singles.tile([1, 1], F32)
        nc.vector.memset(eps_sb, EPS)

        col = [0]

        def take_cols(n):
            c0 = col[0]
            col[0] += n
            return cst[:, c0:c0 + n]

        bi_v = take_cols(D // 128)
        layer_cols = []
        for l in range(NB):
            layer_cols.append(dict(
                bf=take_cols(D // 128),
                b1c=take_cols(DD // 128), g1=take_cols(DD // 128),
                beta1=take_cols(DD // 128), b2c=take_cols(D // 128),
                g2=take_cols(D // 128), beta2=take_cols(D // 128)))
        bout_v = take_cols(CP // 128)

        # ---- generic GEMM driver (swizzled weights, 1 MiB DMAs) ----
        def gemm(wt_dram, k_chunks, m_chunks, rhs_fn, evac_fn, label):
            """psum[m] = sum_k WT[k,m].T @ rhs(k); evac_fn(m, psum)."""
            n_groups = m_chunks // MG
            nkb = k_chunks // KB
            for mg in range(n_groups):
                mlo = mg * MG
                pss = [psum.tile([128, BC], F32, name=f"ps_{label}_{mlo + i}",
                                 tag="ps") for i in range(MG)]
                for kbi in range(nkb):
                    w_sb = wpool.tile([128, KB, MG * 128], F32R,
                                      name=f"w_{label}_{mg}_{kbi}", tag="w")
                    nc.sync.dma_start(out=w_sb, in_=wt_dram[mg, kbi])
                    for kk in range(KB):
                        k = kbi * KB + kk
                        rhs = rhs_fn(k)
                        for i in range(MG):
                            nc.tensor.matmul(
                                pss[i][:, :],
                                lhsT=w_sb[:, kk, i * 128:(i + 1) * 128],
                                rhs=rhs, start=(k == 0),
                                stop=(k == k_chunks - 1))
                for i in range(MG):
                    evac_fn(mlo + i, pss[i])

        # ---- LN helpers ----
        def sq_reduce(ps, bias_col, m, m_chunks, ps_var, label):
            """ps_var[1,BC] += colsum of (psum + bias)^2 (square on ScalarE)."""
            sq = sqpool.tile([128, BC], F32R, name=f"sq_{label}_{m}", tag="sq")
            nc.scalar.activation(out=sq[:, :], in_=ps[:, :], func=AF.Square,
                                 bias=bias_col, scale=1.0)
            nc.tensor.matmul(ps_var[:, :], lhsT=ones_col[:, :], rhs=sq[:, :],
                             start=(m == 0), stop=(m == m_chunks - 1))

        def ln_inv_bcast(ps_var, dsize, label):
            """ibc[128, BC] (PSUM) = broadcast of 1/sqrt(meansq + eps)."""
            std = statpool.tile([1, BC], F32, name=f"std_{label}", tag="std")
            nc.scalar.activation(out=std, in_=ps_var[:, :], func=AF.Sqrt,
                                 bias=eps_sb[:, :], scale=1.0 / dsize)
            inv = statpool.tile([1, BC], F32R, name=f"inv_{label}", tag="inv")
            with nc.allow_low_precision(reason="fp32r LN scale"):
                nc.vector.reciprocal(out=inv, in_=std)
            ibc = pbc.tile([128, BC], F32, name=f"ibc_{label}", tag="bc")
            nc.tensor.matmul(ibc[:, :], lhsT=ones_row[:, :], rhs=inv[:, :],
                             start=True, stop=True)
            return ibc

        # ---- phase 1: h = relu(Wi @ x + bi) ----
        # per-chunk tiles -> chunk-granular dependencies (pipelining across
        # LN barriers); x chunks cycle the same 32 "s" slots the s chunks use.
        x_sb = [bigpool.tile([128, BC], F32R, name=f"x{k}", tag="s", bufs=32)
                for k in range(IN // 128)]
        for k in range(IN // 128):
            nc.sync.dma_start(out=x_sb[k], in_=xt[k * 128:(k + 1) * 128, :])

        def x_chunk(k):
            return x_sb[k][:, :]

        h_a = [hpool.tile([128, BC], F32R, name=f"h_a{m}", tag="ha", bufs=16)
               for m in range(D // 128)]
        h_b = [upool.tile([128, BC], F32R, name=f"h_b{m}", tag="hb", bufs=16)
               for m in range(D // 128)]

        def evac_h0(m, ps):
            nc.scalar.activation(out=h_a[m][:, :], in_=ps[:, :], func=AF.Relu,
                                 bias=bi_v[:, m:m + 1], scale=1.0)

        gemm(wit, IN // 128, D // 128, x_chunk, evac_h0, "wi")

        # ---- phase 2: layers ----
        h_cur = h_a
        h_nxt = h_b
        for l in range(NB):
            lc = layer_cols[l]

            # (a) h_nxt = Wff h_cur + bf   (fused residual attention)
            def evac_att(m, ps, h_nxt=h_nxt, lc=lc):
                nc.scalar.activation(out=h_nxt[m][:, :], in_=ps[:, :],
                                     func=AF.Identity,
                                     bias=lc["bf"][:, m:m + 1], scale=1.0)

            gemm(wfft[l], D // 128, D // 128,
                 lambda k, h=h_cur: h[k][:, :], evac_att, f"wff{l}")

            # (b) s = W1c h_nxt + b1c (pre-centered); var stats on the fly
            s_sb = [bigpool.tile([128, BC], F32R, name=f"s{l}_{m}",
                                 tag="s", bufs=32)
                    for m in range(DD // 128)]
            ps_var1 = pstat.tile([1, BC], F32, name=f"pv1_{l}", tag="pv")

            def evac_s(m, ps, s_sb=s_sb, lc=lc, ps_var1=ps_var1, l=l):
                nc.scalar.activation(out=s_sb[m][:, :], in_=ps[:, :],
                                     func=AF.Identity,
                                     bias=lc["b1c"][:, m:m + 1], scale=1.0)
                sq_reduce(ps, lc["b1c"][:, m:m + 1], m, DD // 128, ps_var1,
                          f"l1_{l}")

            gemm(w1t[l], D // 128, DD // 128,
                 lambda k, h=h_nxt: h[k][:, :], evac_s, f"w1{l}")

            # (c) LN1 + relu, in place on s
            ibc1 = ln_inv_bcast(ps_var1, DD, f"l1_{l}")
            for m in range(DD // 128):
                nc.vector.scalar_tensor_tensor(
                    out=s_sb[m][:, :], in0=s_sb[m][:, :],
                    scalar=lc["g1"][:, m:m + 1], in1=ibc1[:, :],
                    op0=ALU.mult, op1=ALU.mult)
                nc.scalar.activation(out=s_sb[m][:, :], in_=s_sb[m][:, :],
                                     func=AF.Relu,
                                     bias=lc["beta1"][:, m:m + 1], scale=1.0)

            # (d) u = W2c s + b2c (pre-centered); var stats on the fly
            u_mat = [upool.tile([128, BC], F32R, name=f"u{l}_{m}",
                                tag="um", bufs=16) for m in range(D // 128)]
            ps_var2 = pstat.tile([1, BC], F32, name=f"pv2_{l}", tag="pv")

            def evac_u(m, ps, u_mat=u_mat, lc=lc, ps_var2=ps_var2, l=l):
                nc.scalar.activation(out=u_mat[m][:, :], in_=ps[:, :],
                                     func=AF.Identity,
                                     bias=lc["b2c"][:, m:m + 1], scale=1.0)
                sq_reduce(ps, lc["b2c"][:, m:m + 1], m, D // 128, ps_var2,
                          f"l2_{l}")

            gemm(w2t[l], DD // 128, D // 128,
                 lambda k, s=s_sb: s[k][:, :], evac_u, f"w2{l}")

            # (e) LN2 + residual: h_nxt += (u*g2)*ibc2 + beta2  (in place)
            ibc2 = ln_inv_bcast(ps_var2, D, f"l2_{l}")
            for m in range(D // 128):
                nc.vector.scalar_tensor_tensor(
                    out=u_mat[m][:, :], in0=u_mat[m][:, :],
                    scalar=lc["g2"][:, m:m + 1], in1=ibc2[:, :],
                    op0=ALU.mult, op1=ALU.mult)
                nc.vector.scalar_tensor_tensor(
                    out=h_nxt[m][:, :], in0=u_mat[m][:, :],
                    scalar=lc["beta2"][:, m:m + 1], in1=h_nxt[m][:, :],
                    op0=ALU.add, op1=ALU.add)

            h_cur, h_nxt = h_nxt, h_cur

        # ---- phase 3: outT = Wout h + bout ----
        def evac_out(m, ps):
            o_sb = opool.tile([128, BC], F32, name=f"o{m}", tag="o")
            nc.scalar.activation(out=o_sb[:, :], in_=ps[:, :], func=AF.Identity,
                                 bias=bout_v[:, m:m + 1], scale=1.0)
            nc.sync.dma_start(out=out_t[m * 128:(m + 1) * 128, :], in_=o_sb[:, :])

        gemm(woutt, D // 128, CP // 128,
             lambda k, h=h_cur: h[k][:, :], evac_out, "wout")

    nc.compile()
    return nc


def _vec_cols(v):
    v = np.ascontiguousarray(v, dtype=np.float32)
    return v.reshape(-1, 128).T  # [128, L/128]


def _swizzle_w(wt):
    """[K, M] -> [M/512, K/(128*KB), 128, KB*512] so one DMA moves a
    contiguous 1 MiB block with 8 KiB per-partition lines."""
    K, M = wt.shape
    a = wt.reshape(K // (128 * KB), KB, 128, M // 512, 512)
    return np.ascontiguousarray(a.transpose(3, 0, 2, 1, 4).reshape(
        M // 512, K // (128 * KB), 128, KB * 512))


def _swizzle_x(xtc):
    """[IN, BC] -> [IN/(128*KBX), 128, KBX*BC]."""
    a = xtc.reshape(IN // (128 * KBX), KBX, 128, BC)
    return np.ascontiguousarray(a.transpose(0, 2, 1, 3).reshape(
        IN // (128 * KBX), 128, KBX * BC))


def _prep(x, Wi, bi, Wv, bv, Wo, bo, W1, b1, ln1_g, ln1_b,
          W2, b2, ln2_g, ln2_b, Wout, bout):
    f = np.float32
    xt_all = np.ascontiguousarray(np.asarray(x, f).T)          # [IN, B]
    wit = _swizzle_w(np.asarray(Wi, f).T)                      # [IN, D] swz

    shared = {"wit": wit, "consts": None}
    consts_cols = [_vec_cols(np.asarray(bi, f))]
    eye = np.eye(D, dtype=f)
    for l in range(NB):
        Wvl = np.asarray(Wv[l], f)
        Wol = np.asarray(Wo[l], f)
        bvl = np.asarray(bv[l], f)
        bol = np.asarray(bo[l], f)
        Wff = eye + Wol @ Wvl
        bf = Wol @ bvl + bol
        W1l = np.asarray(W1[l], f)
        W2l = np.asarray(W2[l], f)
        W1c = W1l - W1l.mean(axis=0, keepdims=True)
        W2c = W2l - W2l.mean(axis=0, keepdims=True)
        b1l = np.asarray(b1[l], f)
        b2l = np.asarray(b2[l], f)
        shared[f"wfft{l}"] = _swizzle_w(np.ascontiguousarray(Wff.T))
        shared[f"w1t{l}"] = _swizzle_w(np.ascontiguousarray(W1c.T))
        shared[f"w2t{l}"] = _swizzle_w(np.ascontiguousarray(W2c.T))
        consts_cols += [
            _vec_cols(bf),
            _vec_cols(b1l - b1l.mean()), _vec_cols(np.asarray(ln1_g[l], f)),
            _vec_cols(np.asarray(ln1_b[l], f)), _vec_cols(b2l - b2l.mean()),
            _vec_cols(np.asarray(ln2_g[l], f)), _vec_cols(np.asarray(ln2_b[l], f))]
    wout_pad = np.zeros((CP, D), f)
    wout_pad[:C] = np.asarray(Wout, f)
    bout_pad = np.zeros((CP,), f)
    bout_pad[:C] = np.asarray(bout, f)
    shared["woutt"] = _swizzle_w(np.ascontiguousarray(wout_pad.T))
    consts_cols.append(_vec_cols(bout_pad))
    shared["consts"] = np.ascontiguousarray(np.concatenate(consts_cols, axis=1))

    in_maps = []
    for c in range(NCORES):
        m = dict(shared)
        m["xt"] = np.ascontiguousarray(xt_all[:, c * BC:(c + 1) * BC])
        in_maps.append(m)
    return in_maps


def kernel(x, Wi, bi, Wq, bq, Wk, bk, Wv, bv, Wo, bo, conv_w, conv_b,
           W1, b1, ln1_g, ln1_b, W2, b2, ln2_g, ln2_b, Wout, bout):
    # Wq/bq/Wk/bk/conv_w/conv_b are mathematically dead: the model's internal
    # sequence length is 1, so softmax over one key is exactly 1.0 and the
    # attention scores never affect the output.
    global _cached, last_results
    if _cached is None:
        _cached = _build()
    nc = _cached

    in_maps = _prep(x, Wi, bi, Wv, bv, Wo, bo, W1, b1, ln1_g, ln1_b,
                    W2, b2, ln2_g, ln2_b, Wout, bout)
    res = run_bass_kernel_spmd(nc, in_maps, core_ids=list(range(NCORES)))
    last_results = res
    out_t = np.concatenate([r["outT"] for r in res.results], axis=1)  # [CP, B]
    return np.ascontiguousarray(out_t[:C].T)  # [B, C] fp32


# revision 14
# speedup vs baseline: 1.0043x; 1.0043x over previous
"""Trainium2 Bass kernel for nn_EnhancedBioKANModel (dense_transformer).

Model (B=4096, IN=3072, D=2048, C=1000, 3 blocks), with two crucial
mathematical simplifications:

1. The internal sequence length is 1, so attention's softmax over a single
   key is identically 1.0 and the whole score/top-k/conv machinery cancels:
       attn(h) = (h @ Wv^T + bv) @ Wo^T + bo
2. That attention is then a purely linear map, so the residual attention
   block folds into ONE matrix host-side:
       h <- h + attn(h) = (I + Wo Wv) h + (Wo bv + bo) = Wff h + bf

Per layer:   h = Wff h + bf;  z = relu(LN1(W1 h + b1));  h = h + LN2(W2 z + b2)

Strategy: pure data-parallel over batch (512 rows/core on 8 cores),
feature-major activation layout [D partitions, 512 batch free],
host-pre-transposed weights, fp32r matmuls (full PE rate at N=512),
LayerNorm means folded into host-centered W1/W2, variance via
ScalarE-square + PE ones-reduction, per-batch 1/std broadcast via
K=1 matmul.
"""
from contextlib import ExitStack

import numpy as np

import concourse.bacc as bacc
import concourse.bass as bass
import concourse.mybir as mybir
import concourse.tile as tile
from concourse.bass_utils import run_bass_kernel_spmd

F32 = mybir.dt.float32
F32R = mybir.dt.float32r
AF = mybir.ActivationFunctionType
ALU = mybir.AluOpType

NCORES = 8
B, IN, D, DD, C = 4096, 3072, 2048, 4096, 1000
CP = 1024          # padded num_classes
BC = B // NCORES   # 512 batch per core
NB = 3
EPS = 1e-5
MG = 4             # m-chunks per psum group
KB = 4             # k-chunks per weight DMA (1 MiB transfers, 8 KiB lines)
KBX = 8            # k-chunks per x DMA

# consts packing (columns of [128, NCOL] fp32), per layer:
#   bf(16) b1c(32) g1(32) beta1(32) b2c(16) g2(16) beta2(16)
_LAYER_COLS = 160
_NCOL = 16 + NB * _LAYER_COLS + CP // 128

_cached = None
last_results = None


def _build():
    nc = bacc.Bacc(trn_type="TRN2")

    def wparam(name, K, M):
        # swizzled: [M/512 groups, K/(128*KB) blocks, 128 partitions, KB*512]
        return nc.declare_dram_parameter(
            name, [M // (MG * 128), K // (128 * KB), 128, KB * MG * 128],
            F32R, isOutput=False)

    xt = nc.declare_dram_parameter("xt", [IN, BC], F32R, isOutput=False)
    wit = wparam("wit", IN, D)
    wfft = [wparam(f"wfft{l}", D, D) for l in range(NB)]
    w1t = [wparam(f"w1t{l}", D, DD) for l in range(NB)]
    w2t = [wparam(f"w2t{l}", DD, D) for l in range(NB)]
    woutt = wparam("woutt", D, CP)
    consts = nc.declare_dram_parameter("consts", [128, _NCOL], F32, isOutput=False)
    out_t = nc.declare_dram_parameter("outT", [CP, BC], F32, isOutput=True)

    with tile.TileContext(nc) as tc, ExitStack() as ctx:
        wpool = ctx.enter_context(tc.tile_pool(name="w", bufs=3))
        bigpool = ctx.enter_context(tc.tile_pool(name="big", bufs=1))
        hpool = ctx.enter_context(tc.tile_pool(name="h", bufs=1))
        upool = ctx.enter_context(tc.tile_pool(name="u", bufs=1))
        sqpool = ctx.enter_context(tc.tile_pool(name="sq", bufs=3))
        opool = ctx.enter_context(tc.tile_pool(name="o", bufs=2))
        statpool = ctx.enter_context(tc.tile_pool(name="stat", bufs=2))
        singles = ctx.enter_context(tc.tile_pool(name="singles", bufs=1))
        psum = ctx.enter_context(tc.tile_pool(name="psum", bufs=6, space="PSUM"))
        pstat = ctx.enter_context(tc.tile_pool(name="pstat", bufs=1, space="PSUM"))
        pbc = ctx.enter_context(tc.tile_pool(name="pbc", bufs=1, space="PSUM"))

        # ---- constants ----
        cst = singles.tile([128, _NCOL], F32)
        nc.gpsimd.dma_start(out=cst, in_=consts[:, :])

        ones_f = singles.tile([128, 1], F32)
        nc.vector.memset(ones_f, 1.0)
        ones_col = singles.tile([128, 1], F32R)
        nc.vector.tensor_copy(ones_col[:, :], ones_f[:, :])
        ones_row_f = singles.tile([1, 128], F32)
        nc.vector.memset(ones_row_f, 1.0)
        ones_row = singles.tile([1, 128], F32R)
        nc.vector.tensor_copy(ones_row[:, :], ones_row_f[:, :])
        eps_sb = singles.tile([1, 1], F32)
        nc.vector.memset(eps_sb, EPS)

        col = [0]

        def take_cols(n):
            c0 = col[0]
            col[0] += n
            return cst[:, c0:c0 + n]

        bi_v = take_cols(D // 128)
        layer_cols = []
        for l in range(NB):
            layer_cols.append(dict(
                bf=take_cols(D // 128),
                b1c=take_cols(DD // 128), g1=take_cols(DD // 128),
                beta1=take_cols(DD // 128), b2c=take_cols(D // 128),
                g2=take_cols(D // 128), beta2=take_cols(D // 128)))
        bout_v = take_cols(CP // 128)

        # ---- generic GEMM driver (swizzled weights, 1 MiB DMAs) ----
        def gemm(wt_dram, k_chunks, m_chunks, rhs_fn, evac_fn, label):
            """psum[m] = sum_k WT[k,m].T @ rhs(k); evac_fn(m, psum)."""
            n_groups = m_chunks // MG
            nkb = k_chunks // KB
            for mg in range(n_groups):
                mlo = mg * MG
                pss = [psum.tile([128, BC], F32, name=f"ps_{label}_{mlo + i}",
                                 tag="ps") for i in range(MG)]
                for kbi in range(nkb):
                    w_sb = wpool.tile([128, KB, MG * 128], F32R,
                                      name=f"w_{label}_{mg}_{kbi}", tag="w")
                    nc.sync.dma_start(out=w_sb, in_=wt_dram[mg, kbi])
                    for kk in range(KB):
                        k = kbi * KB + kk
                        rhs = rhs_fn(k)
                        for i in range(MG):
                            nc.tensor.matmul(
                                pss[i][:, :],
                                lhsT=w_sb[:, kk, i * 128:(i + 1) * 128],
                                rhs=rhs, start=(k == 0),
                                stop=(k == k_chunks - 1))
                for i in range(MG):
                    evac_fn(mlo + i, pss[i])

        # ---- LN helpers ----
        def sq_reduce(ps, bias_col, m, m_chunks, ps_var, label):
            """ps_var[1,BC] += colsum of (psum + bias)^2 (square on ScalarE)."""
            sq = sqpool.tile([128, BC], F32R, name=f"sq_{label}_{m}", tag="sq")
            nc.scalar.activation(out=sq[:, :], in_=ps[:, :], func=AF.Square,
                                 bias=bias_col, scale=1.0)
            nc.tensor.matmul(ps_var[:, :], lhsT=ones_col[:, :], rhs=sq[:, :],
                             start=(m == 0), stop=(m == m_chunks - 1))

        def ln_inv_bcast(ps_var, dsize, label):
            """ibc[128, BC] (PSUM) = broadcast of 1/sqrt(meansq + eps)."""
            std = statpool.tile([1, BC], F32, name=f"std_{label}", tag="std")
            nc.scalar.activation(out=std, in_=ps_var[:, :], func=AF.Sqrt,
                                 bias=eps_sb[:, :], scale=1.0 / dsize)
            inv = statpool.tile([1, BC], F32R, name=f"inv_{label}", tag="inv")
            with nc.allow_low_precision(reason="fp32r LN scale"):
                nc.vector.reciprocal(out=inv, in_=std)
            ibc = pbc.tile([128, BC], F32, name=f"ibc_{label}", tag="bc")
            nc.tensor.matmul(ibc[:, :], lhsT=ones_row[:, :], rhs=inv[:, :],
                             start=True, stop=True)
            return ibc

        # ---- phase 1: h = relu(Wi @ x + bi) ----
        # per-chunk tiles -> chunk-granular dependencies (pipelining across
        # LN barriers); x chunks cycle the same 32 "s" slots the s chunks use.
        x_sb = [bigpool.tile([128, BC], F32R, name=f"x{k}", tag="s", bufs=32)
                for k in range(IN // 128)]
        for k in range(IN // 128):
            nc.gpsimd.dma_start(out=x_sb[k], in_=xt[k * 128:(k + 1) * 128, :])

        def x_chunk(k):
            return x_sb[k][:, :]

        h_a = [hpool.tile([128, BC], F32R, name=f"h_a{m}", tag="ha", bufs=16)
               for m in range(D // 128)]
        h_b = [upool.tile([128, BC], F32R, name=f"h_b{m}", tag="hb", bufs=16)
               for m in range(D // 128)]

        def evac_h0(m, ps):
            nc.scalar.activation(out=h_a[m][:, :], in_=ps[:, :], func=AF.Relu,
                                 bias=bi_v[:, m:m + 1], scale=1.0)

        gemm(wit, IN // 128, D // 128, x_chunk, evac_h0, "wi")

        # ---- phase 2: layers ----
        h_cur = h_a
        h_nxt = h_b
        for l in range(NB):
            lc = layer_cols[l]

            # (a) h_nxt = Wff h_cur + bf   (fused residual attention)
            def evac_att(m, ps, h_nxt=h_nxt, lc=lc):
                nc.scalar.activation(out=h_nxt[m][:, :], in_=ps[:, :],
                                     func=AF.Identity,
                                     bias=lc["bf"][:, m:m + 1], scale=1.0)

            gemm(wfft[l], D // 128, D // 128,
                 lambda k, h=h_cur: h[k][:, :], evac_att, f"wff{l}")

            # (b) s = W1c h_nxt + b1c (pre-centered); var stats on the fly
            s_sb = [bigpool.tile([128, BC], F32R, name=f"s{l}_{m}",
                                 tag="s", bufs=32)
                    for m in range(DD // 128)]
            ps_var1 = pstat.tile([1, BC], F32, name=f"pv1_{l}", tag="pv")

            def evac_s(m, ps, s_sb=s_sb, lc=lc, ps_var1=ps_var1, l=l):
                nc.scalar.activation(out=s_sb[m][:, :], in_=ps[:, :],
                                     func=AF.Identity,
                                     bias=lc["b1c"][:, m:m + 1], scale=1.0)
                sq_reduce(ps, lc["b1c"][:, m:m + 1], m, DD // 128, ps_var1,
                          f"l1_{l}")

            gemm(w1t[l], D // 128, DD // 128,
                 lambda k, h=h_nxt: h[k][:, :], evac_s, f"w1{l}")

            # (c) LN1 + relu, in place on s
            ibc1 = ln_inv_bcast(ps_var1, DD, f"l1_{l}")
            for m in range(DD // 128):
                nc.vector.scalar_tensor_tensor(
                    out=s_sb[m][:, :], in0=s_sb[m][:, :],
                    scalar=lc["g1"][:, m:m + 1], in1=ibc1[:, :],
                    op0=ALU.mult, op1=ALU.mult)
                nc.scalar.activation(out=s_sb[m][:, :], in_=s_sb[m][:, :],
                                     func=AF.Relu,
                                     bias=lc["beta1"][:, m:m + 1], scale=1.0)

            # (d) u = W2c s + b2c (pre-centered); var stats on the fly
            u_mat = [upool.tile([128, BC], F32R, name=f"u{l}_{m}",
                                tag="um", bufs=16) for m in range(D // 128)]
            ps_var2 = pstat.tile([1, BC], F32, name=f"pv2_{l}", tag="pv")

            def evac_u(m, ps, u_mat=u_mat, lc=lc, ps_var2=ps_var2, l=l):
                nc.scalar.activation(out=u_mat[m][:, :], in_=ps[:, :],
                                     func=AF.Identity,
                                     bias=lc["b2c"][:, m:m + 1], scale=1.0)
                sq_reduce(ps, lc["b2c"][:, m:m + 1], m, D // 128, ps_var2,
                          f"l2_{l}")

            gemm(w2t[l], DD // 128, D // 128,
                 lambda k, s=s_sb: s[k][:, :], evac_u, f"w2{l}")

            # (e) LN2 + residual: h_nxt += (u*g2)*ibc2 + beta2  (in place)
            ibc2 = ln_inv_bcast(ps_var2, D, f"l2_{l}")
            for m in range(D // 128):
                nc.vector.scalar_tensor_tensor(
                    out=u_mat[m][:, :], in0=u_mat[m][:, :],
                    scalar=lc["g2"][:, m:m + 1], in1=ibc2[:, :],
                    op0=ALU.mult, op1=ALU.mult)
                nc.vector.scalar_tensor_tensor(
                    out=h_nxt[m][:, :], in0=u_mat[m][:, :],
                    scalar=lc["beta2"][:, m:m + 1], in1=h_nxt[m][:, :],
                    op0=ALU.add, op1=ALU.add)

            h_cur, h_nxt = h_nxt, h_cur

        # ---- phase 3: outT = Wout h + bout ----
        def evac_out(m, ps):
            o_sb = opool.tile([128, BC], F32, name=f"o{m}", tag="o")
            nc.scalar.activation(out=o_sb[:, :], in_=ps[:, :], func=AF.Identity,
                                 bias=bout_v[:, m:m + 1], scale=1.0)
            nc.gpsimd.dma_start(out=out_t[m * 128:(m + 1) * 128, :], in_=o_sb[:, :])

        gemm(woutt, D // 128, CP // 128,
             lambda k, h=h_cur: h[k][:, :], evac_out, "wout")

    nc.compile()
    return nc


def _vec_cols(v):
    v = np.ascontiguousarray(v, dtype=np.float32)
    return v.reshape(-1, 128).T  # [128, L/128]


def _swizzle_w(wt):
    """[K, M] -> [M/512, K/(128*KB), 128, KB*512] so one DMA moves a
    contiguous 1 MiB block with 8 KiB per-partition lines."""
    K, M = wt.shape
    a = wt.reshape(K // (128 * KB), KB, 128, M // 512, 512)
    return np.ascontiguousarray(a.transpose(3, 0, 2, 1, 4).reshape(
        M // 512, K // (128 * KB), 128, KB * 512))


def _swizzle_x(xtc):
    """[IN, BC] -> [IN/(128*KBX), 128, KBX*BC]."""
    a = xtc.reshape(IN // (128 * KBX), KBX, 128, BC)
    return np.ascontiguousarray(a.transpose(0, 2, 1, 3).reshape(
        IN // (128 * KBX), 128, KBX * BC))


def _prep(x, Wi, bi, Wv, bv, Wo, bo, W1, b1, ln1_g, ln1_b,
          W2, b2, ln2_g, ln2_b, Wout, bout):
    f = np.float32
    xt_all = np.ascontiguousarray(np.asarray(x, f).T)          # [IN, B]
    wit = _swizzle_w(np.asarray(Wi, f).T)                      # [IN, D] swz

    shared = {"wit": wit, "consts": None}
    consts_cols = [_vec_cols(np.asarray(bi, f))]
    eye = np.eye(D, dtype=f)
    for l in range(NB):
        Wvl = np.asarray(Wv[l], f)
        Wol = np.asarray(Wo[l], f)
        bvl = np.asarray(bv[l], f)
        bol = np.asarray(bo[l], f)
        Wff = eye + Wol @ Wvl
        bf = Wol @ bvl + bol
        W1l = np.asarray(W1[l], f)
        W2l = np.asarray(W2[l], f)
        W1c = W1l - W1l.mean(axis=0, keepdims=True)
        W2c = W2l - W2l.mean(axis=0, keepdims=True)
        b1l = np.asarray(b1[l], f)
        b2l = np.asarray(b2[l], f)
        shared[f"wfft{l}"] = _swizzle_w(np.ascontiguousarray(Wff.T))
        shared[f"w1t{l}"] = _swizzle_w(np.ascontiguousarray(W1c.T))
        shared[f"w2t{l}"] = _swizzle_w(np.ascontiguousarray(W2c.T))
        consts_cols += [
            _vec_cols(bf),
            _vec_cols(b1l - b1l.mean()), _vec_cols(np.asarray(ln1_g[l], f)),
            _vec_cols(np.asarray(ln1_b[l], f)), _vec_cols(b2l - b2l.mean()),
            _vec_cols(np.asarray(ln2_g[l], f)), _vec_cols(np.asarray(ln2_b[l], f))]
    wout_pad = np.zeros((CP, D), f)
    wout_pad[:C] = np.asarray(Wout, f)
    bout_pad = np.zeros((CP,), f)
    bout_pad[:C] = np.asarray(bout, f)
    shared["woutt"] = _swizzle_w(np.ascontiguousarray(wout_pad.T))
    consts_cols.append(_vec_cols(bout_pad))
    shared["consts"] = np.ascontiguousarray(np.concatenate(consts_cols, axis=1))

    in_maps = []
    for c in range(NCORES):
        m = dict(shared)
        m["xt"] = np.ascontiguousarray(xt_all[:, c * BC:(c + 1) * BC])
        in_maps.append(m)
    return in_maps


def kernel(x, Wi, bi, Wq, bq, Wk, bk, Wv, bv, Wo, bo, conv_w, conv_b,
           W1, b1, ln1_g, ln1_b, W2, b2, ln2_g, ln2_b, Wout, bout):
    # Wq/bq/Wk/bk/conv_w/conv_b are mathematically dead: the model's internal
    # sequence length is 1, so softmax over one key is exactly 1.0 and the
    # attention scores never affect the output.
    global _cached, last_results
    if _cached is None:
        _cached = _build()
    nc = _cached

    in_maps = _prep(x, Wi, bi, Wv, bv, Wo, bo, W1, b1, ln1_g, ln1_b,
                    W2, b2, ln2_g, ln2_b, Wout, bout)
    res = run_bass_kernel_spmd(nc, in_maps, core_ids=list(range(NCORES)))
    last_results = res
    out_t = np.concatenate([r["outT"] for r in res.results], axis=1)  # [CP, B]
    return np.ascontiguousarray(out_t[:C].T)  # [B, C] fp32


# revision 15
# speedup vs baseline: 1.0113x; 1.0069x over previous
"""Trainium2 Bass kernel for nn_EnhancedBioKANModel (dense_transformer).

Model (B=4096, IN=3072, D=2048, C=1000, 3 blocks), with two crucial
mathematical simplifications:

1. The internal sequence length is 1, so attention's softmax over a single
   key is identically 1.0 and the whole score/top-k/conv machinery cancels:
       attn(h) = (h @ Wv^T + bv) @ Wo^T + bo
2. That attention is then a purely linear map, so the residual attention
   block folds into ONE matrix host-side:
       h <- h + attn(h) = (I + Wo Wv) h + (Wo bv + bo) = Wff h + bf

Per layer:   h = Wff h + bf;  z = relu(LN1(W1 h + b1));  h = h + LN2(W2 z + b2)

Strategy: pure data-parallel over batch (512 rows/core on 8 cores),
feature-major activation layout [D partitions, 512 batch free],
host-pre-transposed weights, fp32r matmuls (full PE rate at N=512),
LayerNorm means folded into host-centered W1/W2, variance via
ScalarE-square + PE ones-reduction, per-batch 1/std broadcast via
K=1 matmul.
"""
from contextlib import ExitStack

import numpy as np

import concourse.bacc as bacc
import concourse.bass as bass
import concourse.mybir as mybir
import concourse.tile as tile
from concourse.bass_utils import run_bass_kernel_spmd

F32 = mybir.dt.float32
F32R = mybir.dt.float32r
AF = mybir.ActivationFunctionType
ALU = mybir.AluOpType

NCORES = 8
B, IN, D, DD, C = 4096, 3072, 2048, 4096, 1000
CP = 1024          # padded num_classes
BC = B // NCORES   # 512 batch per core
NB = 3
EPS = 1e-5
MG = 4             # m-chunks per psum group
KB = 4             # k-chunks per weight DMA (1 MiB transfers, 8 KiB lines)
KBX = 8            # k-chunks per x DMA

# consts packing (columns of [128, NCOL] fp32), per layer:
#   bf(16) b1c(32) g1(32) beta1(32) b2c(16) g2(16) beta2(16)
_LAYER_COLS = 160
_NCOL = 16 + NB * _LAYER_COLS + CP // 128

_cached = None
last_results = None


def _build():
    nc = bacc.Bacc(trn_type="TRN2")

    def wparam(name, K, M):
        # swizzled: [M/512 groups, K/(128*KB) blocks, 128 partitions, KB*512]
        return nc.declare_dram_parameter(
            name, [M // (MG * 128), K // (128 * KB), 128, KB * MG * 128],
            F32R, isOutput=False)

    xt = nc.declare_dram_parameter("xt", [IN, BC], F32R, isOutput=False)
    wit = wparam("wit", IN, D)
    wfft = [wparam(f"wfft{l}", D, D) for l in range(NB)]
    w1t = [wparam(f"w1t{l}", D, DD) for l in range(NB)]
    w2t = [wparam(f"w2t{l}", DD, D) for l in range(NB)]
    woutt = wparam("woutt", D, CP)
    consts = nc.declare_dram_parameter("consts", [128, _NCOL], F32, isOutput=False)
    out_t = nc.declare_dram_parameter("outT", [CP, BC], F32, isOutput=True)

    with tile.TileContext(nc) as tc, ExitStack() as ctx:
        wpool = ctx.enter_context(tc.tile_pool(name="w", bufs=3))
        bigpool = ctx.enter_context(tc.tile_pool(name="big", bufs=1))
        hpool = ctx.enter_context(tc.tile_pool(name="h", bufs=1))
        upool = ctx.enter_context(tc.tile_pool(name="u", bufs=1))
        sqpool = ctx.enter_context(tc.tile_pool(name="sq", bufs=3))
        opool = ctx.enter_context(tc.tile_pool(name="o", bufs=2))
        statpool = ctx.enter_context(tc.tile_pool(name="stat", bufs=2))
        singles = ctx.enter_context(tc.tile_pool(name="singles", bufs=1))
        psum = ctx.enter_context(tc.tile_pool(name="psum", bufs=6, space="PSUM"))
        pstat = ctx.enter_context(tc.tile_pool(name="pstat", bufs=1, space="PSUM"))
        pbc = ctx.enter_context(tc.tile_pool(name="pbc", bufs=1, space="PSUM"))

        # ---- constants ----
        cst = singles.tile([128, _NCOL], F32)
        nc.gpsimd.dma_start(out=cst, in_=consts[:, :])

        ones_f = singles.tile([128, 1], F32)
        nc.vector.memset(ones_f, 1.0)
        ones_col = singles.tile([128, 1], F32R)
        nc.vector.tensor_copy(ones_col[:, :], ones_f[:, :])
        ones_row_f = singles.tile([1, 128], F32)
        nc.vector.memset(ones_row_f, 1.0)
        ones_row = singles.tile([1, 128], F32R)
        nc.vector.tensor_copy(ones_row[:, :], ones_row_f[:, :])
        eps_sb = singles.tile([1, 1], F32)
        nc.vector.memset(eps_sb, EPS)

        col = [0]

        def take_cols(n):
            c0 = col[0]
            col[0] += n
            return cst[:, c0:c0 + n]

        bi_v = take_cols(D // 128)
        layer_cols = []
        for l in range(NB):
            layer_cols.append(dict(
                bf=take_cols(D // 128),
                b1c=take_cols(DD // 128), g1=take_cols(DD // 128),
                beta1=take_cols(DD // 128), b2c=take_cols(D // 128),
                g2=take_cols(D // 128), beta2=take_cols(D // 128)))
        bout_v = take_cols(CP // 128)

        # ---- generic GEMM driver (swizzled weights, 1 MiB DMAs) ----
        def gemm(wt_dram, k_chunks, m_chunks, rhs_fn, evac_fn, label):
            """psum[m] = sum_k WT[k,m].T @ rhs(k); evac_fn(m, psum)."""
            n_groups = m_chunks // MG
            nkb = k_chunks // KB
            for mg in range(n_groups):
                mlo = mg * MG
                pss = [psum.tile([128, BC], F32, name=f"ps_{label}_{mlo + i}",
                                 tag="ps") for i in range(MG)]
                for kbi in range(nkb):
                    w_sb = wpool.tile([128, KB, MG * 128], F32R,
                                      name=f"w_{label}_{mg}_{kbi}", tag="w")
                    nc.sync.dma_start(out=w_sb, in_=wt_dram[mg, kbi])
                    for kk in range(KB):
                        k = kbi * KB + kk
                        rhs = rhs_fn(k)
                        for i in range(MG):
                            nc.tensor.matmul(
                                pss[i][:, :],
                                lhsT=w_sb[:, kk, i * 128:(i + 1) * 128],
                                rhs=rhs, start=(k == 0),
                                stop=(k == k_chunks - 1))
                for i in range(MG):
                    evac_fn(mlo + i, pss[i])

        # ---- LN helpers ----
        def sq_reduce(ps, bias_col, m, m_chunks, ps_var, label):
            """ps_var[1,BC] += colsum of (psum + bias)^2 (square on ScalarE)."""
            sq = sqpool.tile([128, BC], F32R, name=f"sq_{label}_{m}", tag="sq")
            nc.scalar.activation(out=sq[:, :], in_=ps[:, :], func=AF.Square,
                                 bias=bias_col, scale=1.0)
            nc.tensor.matmul(ps_var[:, :], lhsT=ones_col[:, :], rhs=sq[:, :],
                             start=(m == 0), stop=(m == m_chunks - 1))

        def ln_inv_bcast(ps_var, dsize, label):
            """ibc[128, BC] (PSUM) = broadcast of 1/sqrt(meansq + eps)."""
            std = statpool.tile([1, BC], F32, name=f"std_{label}", tag="std")
            nc.scalar.activation(out=std, in_=ps_var[:, :], func=AF.Sqrt,
                                 bias=eps_sb[:, :], scale=1.0 / dsize)
            inv = statpool.tile([1, BC], F32R, name=f"inv_{label}", tag="inv")
            with nc.allow_low_precision(reason="fp32r LN scale"):
                nc.vector.reciprocal(out=inv, in_=std)
            ibc = pbc.tile([128, BC], F32, name=f"ibc_{label}", tag="bc")
            nc.tensor.matmul(ibc[:, :], lhsT=ones_row[:, :], rhs=inv[:, :],
                             start=True, stop=True)
            return ibc

        # ---- phase 1: h = relu(Wi @ x + bi) ----
        # per-chunk tiles -> chunk-granular dependencies (pipelining across
        # LN barriers); x chunks cycle the same 32 "s" slots the s chunks use.
        x_sb = [bigpool.tile([128, BC], F32R, name=f"x{k}", tag="s", bufs=32)
                for k in range(IN // 128)]
        x_loaded = [False] * (IN // 128)

        def x_chunk(k):
            if not x_loaded[k]:
                nc.sync.dma_start(out=x_sb[k], in_=xt[k * 128:(k + 1) * 128, :])
                x_loaded[k] = True
            return x_sb[k][:, :]

        h_a = [hpool.tile([128, BC], F32R, name=f"h_a{m}", tag="ha", bufs=16)
               for m in range(D // 128)]
        h_b = [upool.tile([128, BC], F32R, name=f"h_b{m}", tag="hb", bufs=16)
               for m in range(D // 128)]

        def evac_h0(m, ps):
            nc.scalar.activation(out=h_a[m][:, :], in_=ps[:, :], func=AF.Relu,
                                 bias=bi_v[:, m:m + 1], scale=1.0)

        gemm(wit, IN // 128, D // 128, x_chunk, evac_h0, "wi")

        # ---- phase 2: layers ----
        h_cur = h_a
        h_nxt = h_b
        for l in range(NB):
            lc = layer_cols[l]

            # (a) h_nxt = Wff h_cur + bf   (fused residual attention)
            def evac_att(m, ps, h_nxt=h_nxt, lc=lc):
                nc.scalar.activation(out=h_nxt[m][:, :], in_=ps[:, :],
                                     func=AF.Identity,
                                     bias=lc["bf"][:, m:m + 1], scale=1.0)

            gemm(wfft[l], D // 128, D // 128,
                 lambda k, h=h_cur: h[k][:, :], evac_att, f"wff{l}")

            # (b) s = W1c h_nxt + b1c (pre-centered); var stats on the fly
            s_sb = [bigpool.tile([128, BC], F32R, name=f"s{l}_{m}",
                                 tag="s", bufs=32)
                    for m in range(DD // 128)]
            ps_var1 = pstat.tile([1, BC], F32, name=f"pv1_{l}", tag="pv")

            def evac_s(m, ps, s_sb=s_sb, lc=lc, ps_var1=ps_var1, l=l):
                sq_reduce(ps, lc["b1c"][:, m:m + 1], m, DD // 128, ps_var1,
                          f"l1_{l}")
                nc.scalar.activation(out=s_sb[m][:, :], in_=ps[:, :],
                                     func=AF.Identity,
                                     bias=lc["b1c"][:, m:m + 1], scale=1.0)

            gemm(w1t[l], D // 128, DD // 128,
                 lambda k, h=h_nxt: h[k][:, :], evac_s, f"w1{l}")

            # (c) LN1 + relu, in place on s
            ibc1 = ln_inv_bcast(ps_var1, DD, f"l1_{l}")
            for m in range(DD // 128):
                nc.vector.scalar_tensor_tensor(
                    out=s_sb[m][:, :], in0=s_sb[m][:, :],
                    scalar=lc["g1"][:, m:m + 1], in1=ibc1[:, :],
                    op0=ALU.mult, op1=ALU.mult)
                nc.scalar.activation(out=s_sb[m][:, :], in_=s_sb[m][:, :],
                                     func=AF.Relu,
                                     bias=lc["beta1"][:, m:m + 1], scale=1.0)

            # (d) u = W2c s + b2c (pre-centered); var stats on the fly
            u_mat = [upool.tile([128, BC], F32R, name=f"u{l}_{m}",
                                tag="um", bufs=16) for m in range(D // 128)]
            ps_var2 = pstat.tile([1, BC], F32, name=f"pv2_{l}", tag="pv")

            def evac_u(m, ps, u_mat=u_mat, lc=lc, ps_var2=ps_var2, l=l):
                sq_reduce(ps, lc["b2c"][:, m:m + 1], m, D // 128, ps_var2,
                          f"l2_{l}")
                nc.scalar.activation(out=u_mat[m][:, :], in_=ps[:, :],
                                     func=AF.Identity,
                                     bias=lc["b2c"][:, m:m + 1], scale=1.0)

            gemm(w2t[l], DD // 128, D // 128,
                 lambda k, s=s_sb: s[k][:, :], evac_u, f"w2{l}")

            # (e) LN2 + residual: h_nxt += (u*g2)*ibc2 + beta2  (in place)
            ibc2 = ln_inv_bcast(ps_var2, D, f"l2_{l}")
            for m in range(D // 128):
                nc.vector.scalar_tensor_tensor(
                    out=u_mat[m][:, :], in0=u_mat[m][:, :],
                    scalar=lc["g2"][:, m:m + 1], in1=ibc2[:, :],
                    op0=ALU.mult, op1=ALU.mult)
                nc.vector.scalar_tensor_tensor(
                    out=h_nxt[m][:, :], in0=u_mat[m][:, :],
                    scalar=lc["beta2"][:, m:m + 1], in1=h_nxt[m][:, :],
                    op0=ALU.add, op1=ALU.add)

            h_cur, h_nxt = h_nxt, h_cur

        # ---- phase 3: outT = Wout h + bout ----
        def evac_out(m, ps):
            o_sb = opool.tile([128, BC], F32, name=f"o{m}", tag="o")
            nc.scalar.activation(out=o_sb[:, :], in_=ps[:, :], func=AF.Identity,
                                 bias=bout_v[:, m:m + 1], scale=1.0)
            nc.sync.dma_start(out=out_t[m * 128:(m + 1) * 128, :], in_=o_sb[:, :])

        gemm(woutt, D // 128, CP // 128,
             lambda k, h=h_cur: h[k][:, :], evac_out, "wout")

    nc.compile()
    return nc


def _vec_cols(v):
    v = np.ascontiguousarray(v, dtype=np.float32)
    return v.reshape(-1, 128).T  # [128, L/128]


def _swizzle_w(wt):
    """[K, M] -> [M/512, K/(128*KB), 128, KB*512] so one DMA moves a
    contiguous 1 MiB block with 8 KiB per-partition lines."""
    K, M = wt.shape
    a = wt.reshape(K // (128 * KB), KB, 128, M // 512, 512)
    return np.ascontiguousarray(a.transpose(3, 0, 2, 1, 4).reshape(
        M // 512, K // (128 * KB), 128, KB * 512))


def _swizzle_x(xtc):
    """[IN, BC] -> [IN/(128*KBX), 128, KBX*BC]."""
    a = xtc.reshape(IN // (128 * KBX), KBX, 128, BC)
    return np.ascontiguousarray(a.transpose(0, 2, 1, 3).reshape(
        IN // (128 * KBX), 128, KBX * BC))


def _prep(x, Wi, bi, Wv, bv, Wo, bo, W1, b1, ln1_g, ln1_b,
          W2, b2, ln2_g, ln2_b, Wout, bout):
    f = np.float32
    xt_all = np.ascontiguousarray(np.asarray(x, f).T)          # [IN, B]
    wit = _swizzle_w(np.asarray(Wi, f).T)                      # [IN, D] swz

    shared = {"wit": wit, "consts": None}
    consts_cols = [_vec_cols(np.asarray(bi, f))]
    eye = np.eye(D, dtype=f)
    for l in range(NB):
        Wvl = np.asarray(Wv[l], f)
        Wol = np.asarray(Wo[l], f)
        bvl = np.asarray(bv[l], f)
        bol = np.asarray(bo[l], f)
        Wff = eye + Wol @ Wvl
        bf = Wol @ bvl + bol
        W1l = np.asarray(W1[l], f)
        W2l = np.asarray(W2[l], f)
        W1c = W1l - W1l.mean(axis=0, keepdims=True)
        W2c = W2l - W2l.mean(axis=0, keepdims=True)
        b1l = np.asarray(b1[l], f)
        b2l = np.asarray(b2[l], f)
        shared[f"wfft{l}"] = _swizzle_w(np.ascontiguousarray(Wff.T))
        shared[f"w1t{l}"] = _swizzle_w(np.ascontiguousarray(W1c.T))
        shared[f"w2t{l}"] = _swizzle_w(np.ascontiguousarray(W2c.T))
        consts_cols += [
            _vec_cols(bf),
            _vec_cols(b1l - b1l.mean()), _vec_cols(np.asarray(ln1_g[l], f)),
            _vec_cols(np.asarray(ln1_b[l], f)), _vec_cols(b2l - b2l.mean()),
            _vec_cols(np.asarray(ln2_g[l], f)), _vec_cols(np.asarray(ln2_b[l], f))]
    wout_pad = np.zeros((CP, D), f)
    wout_pad[:C] = np.asarray(Wout, f)
    bout_pad = np.zeros((CP,), f)
    bout_pad[:C] = np.asarray(bout, f)
    shared["woutt"] = _swizzle_w(np.ascontiguousarray(wout_pad.T))
    consts_cols.append(_vec_cols(bout_pad))
    shared["consts"] = np.ascontiguousarray(np.concatenate(consts_cols, axis=1))

    in_maps = []
    for c in range(NCORES):
        m = dict(shared)
        m["xt"] = np.ascontiguousarray(xt_all[:, c * BC:(c + 1) * BC])
        in_maps.append(m)
    return in_maps


def kernel(x, Wi, bi, Wq, bq, Wk, bk, Wv, bv, Wo, bo, conv_w, conv_b,
           W1, b1, ln1_g, ln1_b, W2, b2, ln2_g, ln2_b, Wout, bout):
    # Wq/bq/Wk/bk/conv_w/conv_b are mathematically dead: the model's internal
    # sequence length is 1, so softmax over one key is exactly 1.0 and the
    # attention scores never affect the output.
    global _cached, last_results
    if _cached is None:
        _cached = _build()
    nc = _cached

    in_maps = _prep(x, Wi, bi, Wv, bv, Wo, bo, W1, b1, ln1_g, ln1_b,
                    W2, b2, ln2_g, ln2_b, Wout, bout)
    res = run_bass_kernel_spmd(nc, in_maps, core_ids=list(range(NCORES)))
    last_results = res
    out_t = np.concatenate([r["outT"] for r in res.results], axis=1)  # [CP, B]
    return np.ascontiguousarray(out_t[:C].T)  # [B, C] fp32


# revision 20
# speedup vs baseline: 1.0338x; 1.0222x over previous
"""Trainium2 Bass kernel for nn_EnhancedBioKANModel (dense_transformer).

Model (B=4096, IN=3072, D=2048, C=1000, 3 blocks), with two crucial
mathematical simplifications:

1. The internal sequence length is 1, so attention's softmax over a single
   key is identically 1.0 and the whole score/top-k/conv machinery cancels:
       attn(h) = (h @ Wv^T + bv) @ Wo^T + bo
2. That attention is then a purely linear map, so the residual attention
   block folds into ONE matrix host-side:
       h <- h + attn(h) = (I + Wo Wv) h + (Wo bv + bo) = Wff h + bf

Per layer:   h = Wff h + bf;  z = relu(LN1(W1 h + b1));  h = h + LN2(W2 z + b2)

Strategy: pure data-parallel over batch (512 rows/core on 8 cores),
feature-major activation layout [D partitions, 512 batch free],
host-pre-transposed weights, fp32r matmuls (full PE rate at N=512),
LayerNorm means folded into host-centered W1/W2, variance via
ScalarE-square + PE ones-reduction, per-batch 1/std broadcast via
K=1 matmul.
"""
from contextlib import ExitStack

import numpy as np

import concourse.bacc as bacc
import concourse.bass as bass
import concourse.mybir as mybir
import concourse.tile as tile
from concourse.bass_utils import run_bass_kernel_spmd

F32 = mybir.dt.float32
F32R = mybir.dt.float32r
AF = mybir.ActivationFunctionType
ALU = mybir.AluOpType

NCORES = 8
B, IN, D, DD, C = 4096, 3072, 2048, 4096, 1000
CP = 1024          # padded num_classes
BC = B // NCORES   # 512 batch per core
NB = 3
EPS = 1e-5
MG = 4             # m-chunks per psum group
KB = 4             # k-chunks per weight DMA (1 MiB transfers, 8 KiB lines)
KBX = 8            # k-chunks per x DMA

# consts packing (columns of [128, NCOL] fp32), per layer:
#   bf(16) b1c(32) g1(32) beta1(32) b2c(16) g2(16) beta2(16)
_LAYER_COLS = 160
_NCOL = 16 + NB * _LAYER_COLS + CP // 128

_cached = None
last_results = None


def _build(fast=True):
    nc = bacc.Bacc(trn_type="TRN2")

    def wparam(name, K, M):
        # swizzled: [M/512 groups, K/(128*KB) blocks, 128 partitions, KB*512]
        return nc.declare_dram_parameter(
            name, [M // (MG * 128), K // (128 * KB), 128, KB * MG * 128],
            F32R, isOutput=False)

    xt = nc.declare_dram_parameter("xt", [IN, BC], F32R, isOutput=False)
    wit = wparam("wit", IN, D)
    wfft = [wparam(f"wfft{l}", D, D) for l in range(NB)]
    w1t = [wparam(f"w1t{l}", D, DD) for l in range(NB)]
    w2t = [wparam(f"w2t{l}", DD, D) for l in range(NB)]
    woutt = wparam("woutt", D, CP)
    consts = nc.declare_dram_parameter("consts", [128, _NCOL], F32, isOutput=False)
    out_t = nc.declare_dram_parameter("outT", [CP, BC], F32, isOutput=True)

    with tile.TileContext(nc) as tc, ExitStack() as ctx:
        wpool = ctx.enter_context(tc.tile_pool(name="w", bufs=3))
        bigpool = ctx.enter_context(tc.tile_pool(name="big", bufs=1))
        hpool = ctx.enter_context(tc.tile_pool(name="h", bufs=1))
        upool = ctx.enter_context(tc.tile_pool(name="u", bufs=1))
        sqpool = ctx.enter_context(tc.tile_pool(name="sq", bufs=2))
        opool = ctx.enter_context(tc.tile_pool(name="o", bufs=2))
        statpool = ctx.enter_context(tc.tile_pool(name="stat", bufs=1))
        singles = ctx.enter_context(tc.tile_pool(name="singles", bufs=1))
        psum = ctx.enter_context(tc.tile_pool(name="psum", bufs=6, space="PSUM"))
        pstat = ctx.enter_context(tc.tile_pool(name="pstat", bufs=1, space="PSUM"))
        pbc = ctx.enter_context(tc.tile_pool(name="pbc", bufs=1, space="PSUM"))

        # ---- constants ----
        cst = singles.tile([128, _NCOL], F32)
        nc.gpsimd.dma_start(out=cst, in_=consts[:, :])

        ones_f = singles.tile([128, 1], F32)
        nc.vector.memset(ones_f, 1.0)
        ones_col = singles.tile([128, 1], F32R)
        nc.vector.tensor_copy(ones_col[:, :], ones_f[:, :])
        ones_row_f = singles.tile([1, 128], F32)
        nc.vector.memset(ones_row_f, 1.0)
        ones_row = singles.tile([1, 128], F32R)
        nc.vector.tensor_copy(ones_row[:, :], ones_row_f[:, :])
        eps_sb = singles.tile([1, 1], F32)
        nc.vector.memset(eps_sb, EPS)

        col = [0]

        def take_cols(n):
            c0 = col[0]
            col[0] += n
            return cst[:, c0:c0 + n]

        bi_v = take_cols(D // 128)
        layer_cols = []
        for l in range(NB):
            layer_cols.append(dict(
                bf=take_cols(D // 128),
                b1c=take_cols(DD // 128), g1=take_cols(DD // 128),
                beta1=take_cols(DD // 128), b2c=take_cols(D // 128),
                g2=take_cols(D // 128), beta2=take_cols(D // 128)))
        bout_v = take_cols(CP // 128)

        # ---- generic GEMM driver (swizzled weights, 1 MiB DMAs) ----
        def gemm(wt_dram, k_chunks, m_chunks, rhs_fn, evac_fn, label):
            """psum[m] = sum_k WT[k,m].T @ rhs(k); evac_fn(m, psum)."""
            n_groups = m_chunks // MG
            nkb = k_chunks // KB
            for mg in range(n_groups):
                mlo = mg * MG
                pss = [psum.tile([128, BC], F32, name=f"ps_{label}_{mlo + i}",
                                 tag="ps") for i in range(MG)]
                for kbi in range(nkb):
                    w_sb = wpool.tile([128, KB, MG * 128], F32R,
                                      name=f"w_{label}_{mg}_{kbi}", tag="w")
                    nc.sync.dma_start(out=w_sb, in_=wt_dram[mg, kbi])
                    for kk in range(KB):
                        k = kbi * KB + kk
                        rhs = rhs_fn(k)
                        for i in range(MG):
                            nc.tensor.matmul(
                                pss[i][:, :],
                                lhsT=w_sb[:, kk, i * 128:(i + 1) * 128],
                                rhs=rhs, start=(k == 0),
                                stop=(k == k_chunks - 1))
                for i in range(MG):
                    evac_fn(mlo + i, pss[i])

        # ---- LN helpers ----
        def sq_reduce(ps, bias_col, m, m_chunks, ps_var, label):
            """ps_var[1,BC] += colsum of (psum + bias)^2 (square on ScalarE)."""
            sq = sqpool.tile([128, BC], F32R, name=f"sq_{label}_{m}", tag="sq")
            nc.scalar.activation(out=sq[:, :], in_=ps[:, :], func=AF.Square,
                                 bias=bias_col, scale=1.0)
            nc.tensor.matmul(ps_var[:, :], lhsT=ones_col[:, :], rhs=sq[:, :],
                             start=(m == 0), stop=(m == m_chunks - 1))

        def ln_inv(src_ap, dsize, label, scale=1.0):
            """inv[1,BC] (f32r) = 1/sqrt(src*scale/dsize + eps)."""
            std = statpool.tile([1, BC], F32, name=f"std_{label}", tag="std")
            nc.scalar.activation(out=std, in_=src_ap, func=AF.Sqrt,
                                 bias=eps_sb[:, :], scale=scale / dsize)
            inv = statpool.tile([1, BC], F32R, name=f"inv_{label}", tag="inv",
                                bufs=2)
            with nc.allow_low_precision(reason="fp32r LN scale"):
                nc.vector.reciprocal(out=inv, in_=std)
            return inv

        def bcast(vec, label):
            ibc = pbc.tile([128, BC], F32, name=f"ibc_{label}", tag="bc")
            nc.tensor.matmul(ibc[:, :], lhsT=ones_row[:, :], rhs=vec[:, :],
                             start=True, stop=True)
            return ibc

        # ---- phase 1: h = relu(Wi @ x + bi) ----
        # per-chunk tiles -> chunk-granular dependencies (pipelining across
        # LN barriers); x chunks cycle the same 32 "s" slots the s chunks use.
        x_sb = [bigpool.tile([128, BC], F32R, name=f"x{k}", tag="s", bufs=32)
                for k in range(IN // 128)]
        x_loaded = [False] * (IN // 128)

        def x_chunk(k):
            if not x_loaded[k]:
                nc.sync.dma_start(out=x_sb[k], in_=xt[k * 128:(k + 1) * 128, :])
                x_loaded[k] = True
            return x_sb[k][:, :]

        h_a = [hpool.tile([128, BC], F32R, name=f"h_a{m}", tag="ha", bufs=16)
               for m in range(D // 128)]
        h_b = [upool.tile([128, BC], F32R, name=f"h_b{m}", tag="hb", bufs=16)
               for m in range(D // 128)]

        def evac_h0(m, ps):
            nc.scalar.activation(out=h_a[m][:, :], in_=ps[:, :], func=AF.Relu,
                                 bias=bi_v[:, m:m + 1], scale=1.0)

        gemm(wit, IN // 128, D // 128, x_chunk, evac_h0, "wi")

        # ---- phase 2: layers ----
        h_cur = h_a
        h_nxt = h_b
        for l in range(NB):
            lc = layer_cols[l]

            # (a) h_nxt = Wff h_cur + bf   (fused residual attention)
            def evac_att(m, ps, h_nxt=h_nxt, lc=lc):
                nc.scalar.activation(out=h_nxt[m][:, :], in_=ps[:, :],
                                     func=AF.Identity,
                                     bias=lc["bf"][:, m:m + 1], scale=1.0)

            gemm(wfft[l], D // 128, D // 128,
                 lambda k, h=h_cur: h[k][:, :], evac_att, f"wff{l}")

            # (b) s = W1c h_nxt + b1c (pre-centered); var stats on the fly
            s_sb = [bigpool.tile([128, BC], F32R, name=f"s{l}_{m}",
                                 tag="s", bufs=32)
                    for m in range(DD // 128)]
            ps_var1 = pstat.tile([1, BC], F32, name=f"pv1_{l}", tag="pv")

            def evac_s(m, ps, s_sb=s_sb, lc=lc, ps_var1=ps_var1, l=l):
                sq_reduce(ps, lc["b1c"][:, m:m + 1], m, DD // 128, ps_var1,
                          f"l1_{l}")
                nc.scalar.activation(out=s_sb[m][:, :], in_=ps[:, :],
                                     func=AF.Relu if fast else AF.Identity,
                                     bias=lc["b1c"][:, m:m + 1], scale=1.0)

            gemm(w1t[l], D // 128, DD // 128,
                 lambda k, h=h_nxt: h[k][:, :], evac_s, f"w1{l}")

            # (c) LN1: fast path defers the inv1 column scale through W2
            # (g1 folded into W2 host-side, requires ln1_b == 0);
            # general path applies inv1*g1 + beta1 then relu in place.
            inv1 = ln_inv(ps_var1[:, :], DD, f"l1_{l}")
            if fast:
                inv1sq = statpool.tile([1, BC], F32, name=f"i1sq_{l}",
                                       tag="i1sq")
                nc.vector.tensor_mul(inv1sq[:, :], inv1[:, :], inv1[:, :])
            else:
                ibc1 = bcast(inv1, f"l1_{l}")
                for m in range(DD // 128):
                    nc.vector.scalar_tensor_tensor(
                        out=s_sb[m][:, :], in0=s_sb[m][:, :],
                        scalar=lc["g1"][:, m:m + 1], in1=ibc1[:, :],
                        op0=ALU.mult, op1=ALU.mult)
                    nc.scalar.activation(out=s_sb[m][:, :], in_=s_sb[m][:, :],
                                         func=AF.Relu,
                                         bias=lc["beta1"][:, m:m + 1],
                                         scale=1.0)

            # (d) u = W2c s + b2c (pre-centered); var stats on the fly
            u_mat = [upool.tile([128, BC], F32R, name=f"u{l}_{m}",
                                tag="um", bufs=16) for m in range(D // 128)]
            ps_var2 = pstat.tile([1, BC], F32, name=f"pv2_{l}", tag="pv")

            def evac_u(m, ps, u_mat=u_mat, lc=lc, ps_var2=ps_var2, l=l):
                sq_reduce(ps, lc["b2c"][:, m:m + 1], m, D // 128, ps_var2,
                          f"l2_{l}")
                nc.scalar.activation(out=u_mat[m][:, :], in_=ps[:, :],
                                     func=AF.Identity,
                                     bias=lc["b2c"][:, m:m + 1], scale=1.0)

            gemm(w2t[l], DD // 128, D // 128,
                 lambda k, s=s_sb: s[k][:, :], evac_u, f"w2{l}")

            # (e) LN2 + residual: h_nxt += (u*g2)*ibc2 + beta2  (in place)
            if fast:
                # var2 = inv1^2 * meansq(G); scale applied to h is
                # c12 = inv1 * inv2 (u_true = inv1*G when b2 == 0)
                v2 = statpool.tile([1, BC], F32, name=f"v2_{l}", tag="v2")
                nc.vector.tensor_mul(v2[:, :], inv1sq[:, :], ps_var2[:, :])
                inv2 = ln_inv(v2[:, :], D, f"l2_{l}")
                c12 = statpool.tile([1, BC], F32R, name=f"c12_{l}", tag="c12")
                with nc.allow_low_precision(reason="fp32r LN scale"):
                    nc.vector.tensor_mul(c12[:, :], inv1[:, :], inv2[:, :])
                ibc2 = bcast(c12, f"l2_{l}")
            else:
                ibc2 = bcast(ln_inv(ps_var2[:, :], D, f"l2_{l}"), f"l2b_{l}")
            for m in range(D // 128):
                nc.vector.scalar_tensor_tensor(
                    out=u_mat[m][:, :], in0=u_mat[m][:, :],
                    scalar=lc["g2"][:, m:m + 1], in1=ibc2[:, :],
                    op0=ALU.mult, op1=ALU.mult)
                nc.vector.scalar_tensor_tensor(
                    out=h_nxt[m][:, :], in0=u_mat[m][:, :],
                    scalar=lc["beta2"][:, m:m + 1], in1=h_nxt[m][:, :],
                    op0=ALU.add, op1=ALU.add)

            h_cur, h_nxt = h_nxt, h_cur

        # ---- phase 3: outT = Wout h + bout ----
        def evac_out(m, ps):
            o_sb = opool.tile([128, BC], F32, name=f"o{m}", tag="o")
            nc.scalar.activation(out=o_sb[:, :], in_=ps[:, :], func=AF.Identity,
                                 bias=bout_v[:, m:m + 1], scale=1.0)
            nc.sync.dma_start(out=out_t[m * 128:(m + 1) * 128, :], in_=o_sb[:, :])

        gemm(woutt, D // 128, CP // 128,
             lambda k, h=h_cur: h[k][:, :], evac_out, "wout")

    nc.compile()
    return nc


def _vec_cols(v):
    v = np.ascontiguousarray(v, dtype=np.float32)
    return v.reshape(-1, 128).T  # [128, L/128]


def _swizzle_w(wt):
    """[K, M] -> [M/512, K/(128*KB), 128, KB*512] so one DMA moves a
    contiguous 1 MiB block with 8 KiB per-partition lines."""
    K, M = wt.shape
    a = wt.reshape(K // (128 * KB), KB, 128, M // 512, 512)
    return np.ascontiguousarray(a.transpose(3, 0, 2, 1, 4).reshape(
        M // 512, K // (128 * KB), 128, KB * 512))


def _swizzle_x(xtc):
    """[IN, BC] -> [IN/(128*KBX), 128, KBX*BC]."""
    a = xtc.reshape(IN // (128 * KBX), KBX, 128, BC)
    return np.ascontiguousarray(a.transpose(0, 2, 1, 3).reshape(
        IN // (128 * KBX), 128, KBX * BC))


def _prep(x, Wi, bi, Wv, bv, Wo, bo, W1, b1, ln1_g, ln1_b,
          W2, b2, ln2_g, ln2_b, Wout, bout, fast=True):
    f = np.float32
    xt_all = np.ascontiguousarray(np.asarray(x, f).T)          # [IN, B]
    wit = _swizzle_w(np.asarray(Wi, f).T)                      # [IN, D] swz

    shared = {"wit": wit, "consts": None}
    consts_cols = [_vec_cols(np.asarray(bi, f))]
    eye = np.eye(D, dtype=f)
    for l in range(NB):
        Wvl = np.asarray(Wv[l], f)
        Wol = np.asarray(Wo[l], f)
        bvl = np.asarray(bv[l], f)
        bol = np.asarray(bo[l], f)
        Wff = eye + Wol @ Wvl
        bf = Wol @ bvl + bol
        W1l = np.asarray(W1[l], f)
        W2l = np.asarray(W2[l], f)
        W1c = W1l - W1l.mean(axis=0, keepdims=True)
        W2c = W2l - W2l.mean(axis=0, keepdims=True)
        b1l = np.asarray(b1[l], f)
        b2l = np.asarray(b2[l], f)
        b1cl = b1l - b1l.mean()
        if fast:
            # fold sign(g1) into W1 rows (so relu-at-evac is valid) and
            # |g1| into W2 columns; requires ln1_b == 0.
            g1l = np.asarray(ln1_g[l], f)
            sgn = np.where(g1l < 0, np.float32(-1.0), np.float32(1.0))
            W1c = W1c * sgn[:, None]
            b1cl = b1cl * sgn
            W2c = W2c * np.abs(g1l)[None, :]
        shared[f"wfft{l}"] = _swizzle_w(np.ascontiguousarray(Wff.T))
        shared[f"w1t{l}"] = _swizzle_w(np.ascontiguousarray(W1c.T))
        shared[f"w2t{l}"] = _swizzle_w(np.ascontiguousarray(W2c.T))
        consts_cols += [
            _vec_cols(bf),
            _vec_cols(b1cl), _vec_cols(np.asarray(ln1_g[l], f)),
            _vec_cols(np.asarray(ln1_b[l], f)), _vec_cols(b2l - b2l.mean()),
            _vec_cols(np.asarray(ln2_g[l], f)), _vec_cols(np.asarray(ln2_b[l], f))]
    wout_pad = np.zeros((CP, D), f)
    wout_pad[:C] = np.asarray(Wout, f)
    bout_pad = np.zeros((CP,), f)
    bout_pad[:C] = np.asarray(bout, f)
    shared["woutt"] = _swizzle_w(np.ascontiguousarray(wout_pad.T))
    consts_cols.append(_vec_cols(bout_pad))
    shared["consts"] = np.ascontiguousarray(np.concatenate(consts_cols, axis=1))

    in_maps = []
    for c in range(NCORES):
        m = dict(shared)
        m["xt"] = np.ascontiguousarray(xt_all[:, c * BC:(c + 1) * BC])
        in_maps.append(m)
    return in_maps


def kernel(x, Wi, bi, Wq, bq, Wk, bk, Wv, bv, Wo, bo, conv_w, conv_b,
           W1, b1, ln1_g, ln1_b, W2, b2, ln2_g, ln2_b, Wout, bout):
    # Wq/bq/Wk/bk/conv_w/conv_b are mathematically dead: the model's internal
    # sequence length is 1, so softmax over one key is exactly 1.0 and the
    # attention scores never affect the output.
    global _cached, last_results
    fast = (not np.any(np.asarray(ln1_b)) and not np.any(np.asarray(b2)))
    if _cached is None:
        _cached = {}
    if fast not in _cached:
        _cached[fast] = _build(fast=fast)
    nc = _cached[fast]

    in_maps = _prep(x, Wi, bi, Wv, bv, Wo, bo, W1, b1, ln1_g, ln1_b,
                    W2, b2, ln2_g, ln2_b, Wout, bout, fast=fast)
    res = run_bass_kernel_spmd(nc, in_maps, core_ids=list(range(NCORES)))
    last_results = res
    out_t = np.concatenate([r["outT"] for r in res.results], axis=1)  # [CP, B]
    return np.ascontiguousarray(out_t[:C].T)  # [B, C] fp32
